# revision 1
# baseline (speedup 1.0000x reference)
"""ContrastiveDist kernel for TRN2 (8 NeuronCores, SPMD).

out[n] = sum_e -(t_e . v_n) / (||t_e|| * ||v_n|| + eps)
       = -(s . v_n) / ||v_n||          with s = sum_e t_e / ||t_e||
(eps shifts the result by ~eps/(||t||*||v||) ~ 4e-11 relative -- far
below fp32 noise, so it is dropped.)

Sharding: node_emb split across 8 cores (6250 rows each, padded to
6272 = 49*128); target replicated.  Per-core layout puts node n at
(partition p, tile t) with n = p*49 + t, so tile-windows are contiguous
in DRAM per partition (8KB DMA packets) and the final [128, 49] result
stores with one partition-contiguous DMA.

Phases (emit order = engine FIFO order, chosen so no engine stalls):
  1. target DMA [128,16,256] (entity e = p*16+j, 16KB/partition packets)
     + 7 node chunk DMAs [128,<=8,256].
  2. phase A on target: ACT square -> DVE reduce -> ACT sqrt -> DVE
     reciprocal -> 16 fp32r matmuls (lhsT = 1/||t|| column, rhs = target
     tile; fp32r is 4x fp32 at N=256) accumulating s in PSUM -> ACT copy
     -> GpSimd partition_broadcast -> s_b [128,256].
  3. node ssq: first ACT_CHUNKS chunks per-tile on ACT (Square with
     fused accum_out), rest as batched ACT square + DVE 3D-AP reduce --
     splits the work so DVE and ACT finish together.
  4. dots (after s_b): batched DVE mul (stride-0 broadcast of s_b) +
     reduce(negate) per chunk.
  5. tail: sqrt, reciprocal, multiply, one 25KB store.
"""

import numpy as np
from contextlib import ExitStack

import concourse.bacc as bacc
import concourse.bass as bass
import concourse.mybir as mybir
import concourse.tile as tile
from concourse import bass_utils

E, D = 2048, 256          # entities, embed dim
N_FULL = 50000            # total nodes
N_CORES = 8
NPC = N_FULL // N_CORES   # 6250 true nodes per core
TPC = 49                  # node tiles per core (49*128 = 6272 padded)
NPAD = TPC * 128
ET = E // 128             # 16 entity tiles

CHUNKS = [8, 8, 8, 8, 8, 8, 1]          # node tiles per DMA/compute chunk
ACT_CHUNKS = 3                          # leading chunks: ssq per-tile on ACT

F32 = mybir.dt.float32
F32R = mybir.dt.float32r

_cache = {}


def _build():
    nc = bacc.Bacc(
        "TRN2",
        target_bir_lowering=False,
        debug=False,
        enable_asserts=True,
        num_devices=N_CORES,
    )
    tgt = nc.dram_tensor("target", [E, D], F32, kind="ExternalInput").ap()
    nodes = nc.dram_tensor("nodes", [NPAD, D], F32, kind="ExternalInput").ap()
    out = nc.dram_tensor("out", [NPAD], F32, kind="ExternalOutput").ap()

    with tile.TileContext(nc) as tc, ExitStack() as ctx:
        tpool = ctx.enter_context(tc.tile_pool(name="tgt", bufs=1))
        vpool = ctx.enter_context(tc.tile_pool(name="v", bufs=1))
        spool = ctx.enter_context(tc.tile_pool(name="small", bufs=1))
        scr_pool = ctx.enter_context(tc.tile_pool(name="scr", bufs=2))
        scr2_pool = ctx.enter_context(tc.tile_pool(name="scr2", bufs=2))
        scrA_pool = ctx.enter_context(tc.tile_pool(name="scrA", bufs=2))
        psum = ctx.enter_context(tc.tile_pool(name="psum", bufs=1, space="PSUM"))

        # ---- DMAs first: target (4 pipelined chunks), then node chunks
        TC = 4  # target pipeline chunks of 4 entity-tiles each
        tgt_sb = tpool.tile([128, ET, D], F32)
        tgt_v = tgt.rearrange("(p j) d -> p j d", j=ET)
        for k in range(TC):
            sl = slice(k * (ET // TC), (k + 1) * (ET // TC))
            nc.sync.dma_start(tgt_sb[:, sl, :], tgt_v[:, sl, :])

        nodes_v = nodes.rearrange("(p t) d -> p t d", t=TPC)
        v_tiles = []
        off = 0
        for c, w in enumerate(CHUNKS):
            v = vpool.tile([128, w, D], F32, tag=f"v{c}")
            nc.sync.dma_start(v[:], nodes_v[:, off : off + w, :])
            v_tiles.append((v, off, w))
            off += w

        # ---- phase A: s = sum_e target[e] / ||target[e]|| (entities permuted
        # e = p*16 + j; the sum is permutation invariant).  Fully pipelined
        # per target chunk so the PE matmuls start as early as possible.
        # F32R: rounding-casts on the otherwise-idle GpSimd feed fp32r
        # matmuls (1 cy/row vs 4 for fp32).
        F32R_MM = True
        ssq_t = spool.tile([128, ET], F32)
        inv_tn = spool.tile([128, ET], F32)
        if F32R_MM:
            tgt_r = tpool.tile([128, ET, D], F32R, tag="tgt_r")
            inv_r = spool.tile([128, ET], F32R)
        ps = psum.tile([1, D], F32)
        W = ET // TC
        for k in range(TC):
            sl = slice(k * W, (k + 1) * W)
            scrT = scr2_pool.tile([128, W, D], F32, tag="scrT")
            nc.scalar.activation(
                scrT[:], tgt_sb[:, sl, :], mybir.ActivationFunctionType.Square
            )
            nc.vector.tensor_reduce(
                ssq_t[:, sl], scrT[:],
                axis=mybir.AxisListType.X, op=mybir.AluOpType.add,
            )
            tn_k = spool.tile([128, W], F32, tag=f"tn{k}")
            nc.scalar.sqrt(tn_k[:], ssq_t[:, sl])
            nc.vector.reciprocal(inv_tn[:, sl], tn_k[:])
            if F32R_MM:
                nc.gpsimd.tensor_copy(tgt_r[:, sl, :], tgt_sb[:, sl, :])
                nc.gpsimd.tensor_copy(inv_r[:, sl], inv_tn[:, sl])
                mm_w, mm_x = inv_r, tgt_r
            else:
                mm_w, mm_x = inv_tn, tgt_sb
            for j in range(k * W, (k + 1) * W):
                nc.tensor.matmul(
                    ps[:],
                    mm_w[:, j : j + 1],
                    mm_x[:, j, :],
                    start=(j == 0),
                    stop=(j == ET - 1),
                )
        s_row = spool.tile([1, D], F32)
        nc.vector.tensor_copy(s_row[:], ps[:])
        s_b = spool.tile([128, D], F32)
        nc.gpsimd.partition_broadcast(s_b[:], s_row[:])

        # ---- node ssq (independent of s_b; starts as soon as data lands)
        ssq_v = spool.tile([128, TPC], F32)
        for c, (v, off, w) in enumerate(v_tiles):
            if c < ACT_CHUNKS:
                # per-tile Square with fused accumulate on ACT
                for j in range(w):
                    scrA = scrA_pool.tile([128, D], F32)
                    nc.scalar.activation(
                        scrA[:], v[:, j, :], mybir.ActivationFunctionType.Square,
                        accum_out=ssq_v[:, off + j : off + j + 1],
                    )
            else:
                scr2 = scr2_pool.tile([128, w, D], F32)
                nc.scalar.activation(
                    scr2[:], v[:], mybir.ActivationFunctionType.Square
                )
                nc.vector.tensor_reduce(
                    ssq_v[:, off : off + w], scr2[:],
                    axis=mybir.AxisListType.X, op=mybir.AluOpType.add,
                )

        # ---- dots: -v.s per node (DVE, batched; s_b broadcast at stride 0)
        negdot = spool.tile([128, TPC], F32)
        for c, (v, off, w) in enumerate(v_tiles):
            scr = scr_pool.tile([128, w, D], F32)
            nc.vector.tensor_mul(
                scr[:], v[:], s_b[:].unsqueeze(1).broadcast_to([128, w, D])
            )
            nc.vector.tensor_reduce(
                negdot[:, off : off + w], scr[:],
                axis=mybir.AxisListType.X, op=mybir.AluOpType.add, negate=True,
            )

        # ---- tail
        vn = spool.tile([128, TPC], F32)
        nc.scalar.sqrt(vn[:], ssq_v[:])
        inv_vn = spool.tile([128, TPC], F32)
        nc.vector.reciprocal(inv_vn[:], vn[:])
        res = spool.tile([128, TPC], F32)
        nc.vector.tensor_mul(res[:], negdot[:], inv_vn[:])
        nc.sync.dma_start(out.rearrange("(p t) -> p t", t=TPC), res[:])

    nc.compile()
    return nc


def _get_nc():
    if "nc" not in _cache:
        _cache["nc"] = _build()
    return _cache["nc"]


def run(pred, target, node_emb, trace=False, **trace_kwargs):
    """Returns (full_output [50000] f32, BassKernelResults)."""
    target = np.ascontiguousarray(np.asarray(target, dtype=np.float32))
    node_emb = np.ascontiguousarray(np.asarray(node_emb, dtype=np.float32))

    nc = _get_nc()
    in_maps = []
    for c in range(N_CORES):
        shard = np.empty((NPAD, D), dtype=np.float32)
        shard[:NPC] = node_emb[c * NPC : (c + 1) * NPC]
        shard[NPC:] = node_emb[: NPAD - NPC]  # pad with real rows (no 0-norm)
        in_maps.append({"target": target, "nodes": shard})

    res = bass_utils.run_bass_kernel_spmd(
        nc, in_maps, list(range(N_CORES)), trace=trace, **trace_kwargs
    )
    parts = [res.results[c]["out"][:NPC] for c in range(N_CORES)]
    return np.concatenate(parts).astype(np.float32), res


def kernel(pred, target, node_emb):
    out, _ = run(pred, target, node_emb)
    return out



# revision 15
# speedup vs baseline: 1.7182x; 1.7182x over previous
"""ContrastiveDist kernel for TRN2 (8 NeuronCores, SPMD) -- v2.

out[n] = sum_e -(t_e . v_n) / (||t_e|| * ||v_n|| + eps)
       = -(s . v_n) / ||v_n||          with s = sum_e t_e / ||t_e||
(eps shifts the result by ~4e-11 relative -- dropped.)

v2 design (vs v1 which was DVE-bound at ~68us):
 * Everything ships as bf16 (host-side cast): halves HBM traffic, and the
   2e-2 rel-err budget has ~10x margin over bf16 noise (~2e-3).
 * node_emb is shipped TRANSPOSED per shard ([256 d, 6272 n], d on
   partitions).  With d on the partition axis the PE (matmul) does both
   per-node reductions -- dots via lhsT=s column, ssq via lhsT=ones
   against elementwise-squared v -- instead of DVE mul+reduce.  Each
   448-col group g accumulates into psum partition g, so the result
   lands as [14, 448] (nodes = g*448+f) and the tail is lane-parallel.
 * DVE only squares (bf16 packed 2x/cycle) + tiny phase-A ops.
 * ACT holds ONE table set the whole kernel (sqrt_and_others contains
   both sqrt and the square filler), loaded at t=0 by a dummy sqrt.
 * PE pre-warmed with dummy matmuls so phase A + node matmuls run at
   2.4GHz (HAM clock gate needs ~3.4us of sustained activity).
 * DMA (HWDGE FIFO): target first (2 chunks), then node chunks sized
   [1,3,3,3,2,1,1]x448 cols -- small first chunk starts PE early, small
   last chunks shorten the drain tail.
"""

import numpy as np
import ml_dtypes
from contextlib import ExitStack

import concourse.bacc as bacc
import concourse.bass as bass
import concourse.mybir as mybir
import concourse.tile as tile
from concourse import bass_utils

E, D = 2048, 256          # entities, embed dim
N_FULL = 50000            # total nodes
N_CORES = 8
NPC = N_FULL // N_CORES   # 6250 true nodes per core
G = 448                   # node columns per psum group (fp32 bank width)
NG = 14                   # groups per core -> NPAD = 6272
NPAD = G * NG
A = 2                     # d-halves (256 = 2*128 partitions)
ET = E // 128             # 16 entity tiles
TCH = 2                   # target DMA chunks
VCH = [1, 3, 3, 3, 2, 1, 1]   # node chunk sizes in groups (sum = NG)
SQ_ENG = "VVVVVVV"        # per-chunk square engine: V=vector, S=scalar
WARM_MM = 8               # PE prewarm dummy matmuls (~370ns each cold)

F32 = mybir.dt.float32
BF16 = mybir.dt.bfloat16
BF = ml_dtypes.bfloat16

_cache = {}


def _build():
    nc = bacc.Bacc(
        "TRN2",
        target_bir_lowering=False,
        debug=False,
        enable_asserts=True,
        num_devices=N_CORES,
    )
    tgt = nc.dram_tensor("target", [E, D], BF16, kind="ExternalInput").ap()
    vt = nc.dram_tensor("vt", [D, NPAD], BF16, kind="ExternalInput").ap()
    # eye[p, g, m] = (g == m): selector columns for the block-diagonal
    # lhsT trick (PE out base partition must be 0/32/64, so group g is
    # routed to psum row g via lhsT column placement instead).
    eye = nc.dram_tensor("eye", [128, NG * NG], BF16, kind="ExternalInput").ap()
    out = nc.dram_tensor("out", [NPAD], F32, kind="ExternalOutput").ap()

    with tile.TileContext(nc) as tc, ExitStack() as ctx:
        tpool = ctx.enter_context(tc.tile_pool(name="tgt", bufs=1))
        vpool = ctx.enter_context(tc.tile_pool(name="v", bufs=1))
        spool = ctx.enter_context(tc.tile_pool(name="small", bufs=1))
        scr = ctx.enter_context(tc.tile_pool(name="scr", bufs=1))
        ps_w = ctx.enter_context(tc.tile_pool(name="psw", bufs=1, space="PSUM"))
        ps_s0 = ctx.enter_context(tc.tile_pool(name="pss0", bufs=1, space="PSUM"))
        ps_s1 = ctx.enter_context(tc.tile_pool(name="pss1", bufs=1, space="PSUM"))
        ps_d = ctx.enter_context(tc.tile_pool(name="psd", bufs=1, space="PSUM"))
        ps_q = ctx.enter_context(tc.tile_pool(name="psq", bufs=1, space="PSUM"))

        tgt_sb = tpool.tile([128, ET, D], BF16)
        tsq = scr.tile([128, ET, D], BF16, tag="tsq")
        vt_sb = vpool.tile([128, A, NPAD], BF16, tag="vt")
        vsq = vpool.tile([128, A, NPAD], BF16, tag="vsq")

        ssq_t = spool.tile([128, ET], F32, tag="ssqt")
        tn = spool.tile([128, ET], F32, tag="tn")
        inv_t = spool.tile([128, ET], F32, tag="invt")
        inv_bf = spool.tile([128, ET], BF16, tag="invbf")
        eye_sb = spool.tile([128, NG, NG], BF16, tag="eye")
        s_bf = spool.tile([128, A], BF16, tag="sbf")
        dotw = spool.tile([128, A, NG, NG], BF16, tag="dotw")
        warm_w = spool.tile([128, 1], BF16, tag="warmw")
        warm_x = spool.tile([128, G], BF16, tag="warmx")
        act_d = spool.tile([1, 1], F32, tag="actd")
        act_s = spool.tile([1, 1], F32, tag="acts")
        vn = spool.tile([NG, G], F32, tag="vn")
        isv = spool.tile([NG, G], F32, tag="isv")
        res = spool.tile([NG, G], F32, tag="res")

        warm_ps = ps_w.tile([1, G], F32)
        # one bank per d-half: a start=True in a bank clears the whole
        # bank, so the two halves' accumulation groups must not share.
        s_ps = [
            ps_s0.tile([128, 1], F32, name="sps0"),
            ps_s1.tile([128, 1], F32, name="sps1"),
        ]
        dot_ps = ps_d.tile([NG, G], F32)
        sq_ps = ps_q.tile([NG, G], F32)

        tgt_v = tgt.rearrange("(p j) d -> p j d", j=ET)
        vt_v = vt.rearrange("(a p) n -> p a n", p=128)
        out_v = out.rearrange("(g f) -> g f", f=G)

        # ---- DMAs (HWDGE FIFO: emission order = arrival order)
        nc.sync.dma_start(eye_sb[:], eye.rearrange("p (g m) -> p g m", m=NG))
        H = ET // TCH
        for k in range(TCH):
            nc.sync.dma_start(
                tgt_sb[:, k * H : (k + 1) * H, :], tgt_v[:, k * H : (k + 1) * H, :]
            )
        chunks = []
        gb = 0
        for w in VCH:
            c0, c1 = gb * G, (gb + w) * G
            nc.sync.dma_start(vt_sb[:, :, c0:c1], vt_v[:, :, c0:c1])
            chunks.append((gb, w))
            gb += w

        # ---- consts
        nc.vector.memset(warm_w[:], 1.0)
        nc.vector.memset(warm_x[:], 0.0)
        nc.vector.memset(act_d[:], 1.0)

        # ---- ACT table preload (sqrt_and_others has sqrt + square filler)
        nc.scalar.sqrt(act_s[:], act_d[:])

        # ---- PE prewarm (HAM clock-gate: ~3.4us busy -> 2.4GHz)
        for _ in range(WARM_MM):
            nc.tensor.matmul(warm_ps[:], warm_w[:], warm_x[:], start=True, stop=True)

        # ---- phase A: s = sum_e t_e/||t_e||, landed directly as lhsT
        # columns s_a [128, 1] (d = a*128 + p), negated so the tail is a
        # plain multiply.
        for k in range(TCH):
            sl = slice(k * H, (k + 1) * H)
            nc.vector.tensor_mul(tsq[:, sl, :], tgt_sb[:, sl, :], tgt_sb[:, sl, :])
            nc.vector.tensor_reduce(
                ssq_t[:, sl], tsq[:, sl, :],
                axis=mybir.AxisListType.X, op=mybir.AluOpType.add,
            )
            nc.scalar.sqrt(tn[:, sl], ssq_t[:, sl])
            nc.vector.reciprocal(inv_t[:, sl], tn[:, sl])
            nc.vector.tensor_scalar_mul(inv_bf[:, sl], inv_t[:, sl], -1.0)
            for j in range(k * H, (k + 1) * H):
                for a in range(A):
                    nc.tensor.matmul(
                        s_ps[a][:],
                        tgt_sb[:, j, a * 128 : (a + 1) * 128],
                        inv_bf[:, j : j + 1],
                        start=(j == 0),
                        stop=(j == ET - 1),
                    )
        for a in range(A):
            nc.vector.tensor_copy(s_bf[:, a : a + 1], s_ps[a][:])
        # dotw[p, a, g, m] = s_a[p] * (g == m)
        for a in range(A):
            nc.vector.tensor_mul(
                dotw[:, a], eye_sb[:],
                s_bf[:, a : a + 1].unsqueeze(2).broadcast_to([128, NG, NG]),
            )

        # ---- node chunks: square (DVE/ACT), then PE reduces per group.
        # lhsT column g routes group g's reduction to psum row g; the
        # other 13 rows accumulate zeros.  One accumulation group spans
        # all 28 matmuls per psum tile.
        for ci, (g0, w) in enumerate(chunks):
            c0, c1 = g0 * G, (g0 + w) * G
            if SQ_ENG[ci] == "S":
                nc.scalar.activation(
                    vsq[:, :, c0:c1], vt_sb[:, :, c0:c1],
                    mybir.ActivationFunctionType.Square,
                )
            else:
                nc.vector.tensor_mul(
                    vsq[:, :, c0:c1], vt_sb[:, :, c0:c1], vt_sb[:, :, c0:c1]
                )
            for g in range(g0, g0 + w):
                for a in range(A):
                    nc.tensor.matmul(
                        sq_ps[:, :],
                        eye_sb[:, g, :],
                        vsq[:, a, g * G : (g + 1) * G],
                        start=(g == 0 and a == 0),
                        stop=(g == NG - 1 and a == 1),
                    )
            for g in range(g0, g0 + w):
                for a in range(A):
                    nc.tensor.matmul(
                        dot_ps[:, :],
                        dotw[:, a, g, :],
                        vt_sb[:, a, g * G : (g + 1) * G],
                        start=(g == 0 and a == 0),
                        stop=(g == NG - 1 and a == 1),
                    )

        # ---- tail: out = (-s.v) / sqrt(ssq), split along the free axis
        # (PSUM reads must start at a 32-aligned partition, so no
        # partition split).
        for f0, f1 in [(0, G // 2), (G // 2, G)]:
            f = slice(f0, f1)
            nc.scalar.sqrt(vn[:, f], sq_ps[:, f])
            nc.vector.reciprocal(isv[:, f], vn[:, f])
            nc.vector.tensor_mul(res[:, f], dot_ps[:, f], isv[:, f])
            nc.sync.dma_start(out_v[:, f], res[:, f])

    nc.compile()
    return nc


def _get_nc():
    if "nc" not in _cache:
        _cache["nc"] = _build()
    return _cache["nc"]


def _eye():
    if "eye" not in _cache:
        e = np.zeros((128, NG, NG), dtype=BF)
        for g in range(NG):
            e[:, g, g] = 1.0
        _cache["eye"] = np.ascontiguousarray(e.reshape(128, NG * NG))
    return _cache["eye"]


def run(pred, target, node_emb, trace=False, **trace_kwargs):
    """Returns (full_output [50000] f32, BassKernelResults)."""
    target = np.asarray(target, dtype=np.float32)
    node_emb = np.asarray(node_emb, dtype=np.float32)
    tgt_bf = np.ascontiguousarray(target).astype(BF)

    nc = _get_nc()
    in_maps = []
    for c in range(N_CORES):
        shard = np.empty((NPAD, D), dtype=np.float32)
        shard[:NPC] = node_emb[c * NPC : (c + 1) * NPC]
        shard[NPC:] = node_emb[: NPAD - NPC]  # pad with real rows (no 0-norm)
        vt = shard.T.astype(BF, order="C")    # [256, 6272] bf16, d-major
        in_maps.append({"target": tgt_bf, "vt": vt, "eye": _eye()})

    res = bass_utils.run_bass_kernel_spmd(
        nc, in_maps, list(range(N_CORES)), trace=trace, **trace_kwargs
    )
    parts = [res.results[c]["out"][:NPC] for c in range(N_CORES)]
    return np.concatenate(parts).astype(np.float32), res


def kernel(pred, target, node_emb):
    out, _ = run(pred, target, node_emb)
    return out
